# revision 5
# baseline (speedup 1.0000x reference)
"""KVCache prefill quantize+scatter kernel for 8 Trainium2 NeuronCores.

Reference semantics (see problem): for key/value [B=4,S=1024,H=32,D=128] f32:
  scale = max|x| over D                          -> [B,S,H,1]
  q     = rint(x * (127.5/scale)) as int8 (saturating, round-half-even)
  both transposed to cache layout (s,h,b,d) and scattered into
  cached_* buffers of shape [S_MAX=2048,...] at offset 0.

Sharding: the S axis is split across the 8 cores (128 rows each).  Every
core's input slice key[:, c*128:(c+1)*128] is 4 contiguous 2 MiB blocks and
its output slice cached_key[c*128:(c+1)*128] is one contiguous 2 MiB block,
so all DMAs are fully contiguous and no inter-core communication is needed.
Cache rows [1024:2048] are a pass-through of the input cache (host-side).

Numerics: on this platform the jax reference's `127.5/scale` lowers to
reciprocal+multiply (verified empirically: reference == rint(x *
fl(127.5*fl(1/s))) bit-exactly, NOT the IEEE division).  The DVE
InstReciprocal is bit-exact fl(1/s), so t = tensor_scalar(r, 127.5, mult)
reproduces the reference scale exactly.  The HW f32->int8 cast rounds
half-to-even and saturates, matching jnp.rint().astype(int8) on XLA.
(An exact-division Markstein/Dekker chain was validated on HW too — see
_exact_div_chain — but the platform reference is the reciprocal path.)
"""

import os
import numpy as np
import ml_dtypes
from contextlib import ExitStack

B, S, H, D = 4, 1024, 32, 128
S_MAX = 2048
N_CORES = 8
S_SH = S // N_CORES          # 128 cache rows per core
HD = H * D                   # 4096
HB = H * B                   # 128
MAX_INT8 = 127.5
DEKKER_C = 4097.0            # 2^12 + 1

_cached = {}


def _exact_div_chain(nc, mybir, pool, s_ap, t_tile):
    """t = fl(127.5 / s) elementwise, bit-exact vs IEEE f32 division.

    s_ap/t_tile: [128, G] f32 SBUF APs.  Uses the exact DVE reciprocal,
    then corrects the q0 = fl(127.5*r) candidate with e = 127.5 - q0*s
    computed exactly (Dekker 2-prod for q0*s).
    """
    tt = nc.vector.tensor_tensor
    ts = nc.vector.tensor_scalar
    mul = mybir.AluOpType.mult
    sub = mybir.AluOpType.subtract
    add = mybir.AluOpType.add
    G = s_ap.shape[1]
    f32 = mybir.dt.float32

    def tmp(tag):
        return pool.tile([128, G], f32, tag=tag, name=tag)

    r = tmp("r"); q0 = tmp("q0")
    nc.vector.reciprocal(r[:], s_ap)
    ts(q0[:], r[:], MAX_INT8, None, op0=mul)
    # Dekker split of q0 and s
    c1 = tmp("c1"); d1 = tmp("d1"); qh = tmp("qh"); ql = tmp("ql")
    ts(c1[:], q0[:], DEKKER_C, None, op0=mul)
    tt(d1[:], c1[:], q0[:], op=sub)
    tt(qh[:], c1[:], d1[:], op=sub)
    tt(ql[:], q0[:], qh[:], op=sub)
    c2 = tmp("c2"); d2 = tmp("d2"); sh = tmp("sh"); sl = tmp("sl")
    ts(c2[:], s_ap, DEKKER_C, None, op0=mul)
    tt(d2[:], c2[:], s_ap, op=sub)
    tt(sh[:], c2[:], d2[:], op=sub)
    tt(sl[:], s_ap, sh[:], op=sub)
    # p = fl(q0*s); err = q0*s - p exactly
    p = tmp("p"); e1 = tmp("e1"); e2 = tmp("e2")
    tt(p[:], q0[:], s_ap, op=mul)
    tt(e1[:], qh[:], sh[:], op=mul)
    tt(e2[:], e1[:], p[:], op=sub)
    tt(e1[:], qh[:], sl[:], op=mul)
    tt(e2[:], e2[:], e1[:], op=add)
    tt(e1[:], ql[:], sh[:], op=mul)
    tt(e2[:], e2[:], e1[:], op=add)
    tt(e1[:], ql[:], sl[:], op=mul)
    tt(e2[:], e2[:], e1[:], op=add)          # e2 = err
    # e_neg = (p - 127.5) + err = -(127.5 - q0*s)
    en = tmp("en")
    nc.vector.scalar_tensor_tensor(en[:], p[:], MAX_INT8, e2[:], op0=sub, op1=add)
    # t = q0 - fl(e_neg * r) = q0 + e*r
    fn = tmp("fn")
    tt(fn[:], en[:], r[:], op=mul)
    tt(t_tile[:], q0[:], fn[:], op=sub)


def _build():
    import concourse.bass as bass
    import concourse.tile as tile
    from concourse import bacc, mybir

    nc = bacc.Bacc("TRN2", target_bir_lowering=False, debug=False,
                   num_devices=N_CORES)
    f32, i8, bf16 = mybir.dt.float32, mybir.dt.int8, mybir.dt.bfloat16

    ins = {
        "k_in": nc.dram_tensor("k_in", [B, S_SH, H, D], f32,
                               kind="ExternalInput").ap(),
        "v_in": nc.dram_tensor("v_in", [B, S_SH, H, D], f32,
                               kind="ExternalInput").ap(),
    }
    ck = nc.dram_tensor("ck", [S_SH, H, B, D], i8, kind="ExternalOutput").ap()
    cv = nc.dram_tensor("cv", [S_SH, H, B, D], i8, kind="ExternalOutput").ap()
    cks = nc.dram_tensor("cks", [S_SH, H, B], bf16, kind="ExternalOutput").ap()
    cvs = nc.dram_tensor("cvs", [S_SH, H, B], bf16, kind="ExternalOutput").ap()

    with tile.TileContext(nc) as tc:
        with ExitStack() as ctx:
            xpool = ctx.enter_context(tc.tile_pool(name="x", bufs=6))
            qpool = ctx.enter_context(tc.tile_pool(name="q", bufs=2))
            spool = ctx.enter_context(tc.tile_pool(name="s", bufs=2))

            for x_in, q_out, s_out in ((ins["k_in"], ck, cks),
                                       (ins["v_in"], cv, cvs)):
                xs = []
                s_all = spool.tile([128, HB], f32, tag="s_all")
                s3 = s_all[:].rearrange("p (h b) -> p h b", h=H)
                for b in range(B):
                    xb = xpool.tile([128, HD], f32, tag="xb")
                    nc.sync.dma_start(
                        xb[:], x_in[b].rearrange("p h d -> p (h d)"))
                    xs.append(xb)
                    nc.vector.reduce_max(
                        s3[:, :, b], xb[:].rearrange("p (h d) -> p h d", h=H),
                        axis=mybir.AxisListType.X, apply_absolute_value=True)
                t_all = spool.tile([128, HB], f32, tag="t_all")
                r_all = spool.tile([128, HB], f32, tag="r_all")
                nc.vector.reciprocal(r_all[:], s_all[:])
                nc.vector.tensor_scalar(t_all[:], r_all[:], MAX_INT8, None,
                                        op0=mybir.AluOpType.mult)
                sb16 = spool.tile([128, HB], bf16, tag="sb16")
                nc.vector.tensor_copy(sb16[:], s_all[:])
                nc.sync.dma_start(s_out.rearrange("p h b -> p (h b)"), sb16[:])
                t3 = t_all[:].rearrange("p (h b) -> p h b", h=H)
                q_all = qpool.tile([128, H * B * D], i8, tag="q_all")
                q4 = q_all[:].rearrange("p (h b d) -> p h b d", h=H, b=B)
                for b in range(B):
                    nc.vector.tensor_tensor(
                        q4[:, :, b, :],
                        xs[b][:].rearrange("p (h d) -> p h d", h=H),
                        t3[:, :, b].unsqueeze(2).broadcast_to([128, H, D]),
                        op=mybir.AluOpType.mult)
                nc.sync.dma_start(
                    q_out.rearrange("p h b d -> p (h b d)"), q_all[:])

    nc.compile()
    return nc


def kernel(key, value, cached_key, cached_value, cached_key_scale,
           cached_value_scale):
    key = np.asarray(key)
    value = np.asarray(value)
    cached_key = np.asarray(cached_key)
    cached_value = np.asarray(cached_value)
    cached_key_scale = np.asarray(cached_key_scale)
    cached_value_scale = np.asarray(cached_value_scale)

    if "nc" not in _cached:
        _cached["nc"] = _build()
    nc = _cached["nc"]

    in_maps = []
    for c in range(N_CORES):
        sl = slice(c * S_SH, (c + 1) * S_SH)
        in_maps.append({
            "k_in": np.ascontiguousarray(key[:, sl]),
            "v_in": np.ascontiguousarray(value[:, sl]),
        })

    from concourse import bass_utils
    res = bass_utils.run_bass_kernel_spmd(
        nc, in_maps, core_ids=list(range(N_CORES)),
        trace=bool(os.environ.get("KERNEL_TRACE")))
    _cached["last_results"] = res

    new_ck = np.empty((S_MAX, H, B, D), np.int8)
    new_cv = np.empty((S_MAX, H, B, D), np.int8)
    new_cks = np.empty((S_MAX, H, B, 1), ml_dtypes.bfloat16)
    new_cvs = np.empty((S_MAX, H, B, 1), ml_dtypes.bfloat16)
    new_ck[S:] = cached_key[S:]
    new_cv[S:] = cached_value[S:]
    new_cks[S:] = cached_key_scale[S:]
    new_cvs[S:] = cached_value_scale[S:]
    for c in range(N_CORES):
        sl = slice(c * S_SH, (c + 1) * S_SH)
        out = res.results[c]
        new_ck[sl] = out["ck"]
        new_cv[sl] = out["cv"]
        new_cks[sl] = out["cks"].reshape(S_SH, H, B, 1)
        new_cvs[sl] = out["cvs"].reshape(S_SH, H, B, 1)
    return new_ck, new_cks, new_cv, new_cvs
